# revision 31
# baseline (speedup 1.0000x reference)
"""Routed MoE classifier head for Trainium2 (8 NeuronCores, SPMD).

The reference computes all 8 experts densely and selects; here each sample is
routed to exactly one expert.  On the host we gather samples by expert
(expert e -> core e), pad to a common S, and pre-transpose x so the
contraction dim D lands on SBUF partitions.  Each core runs a dense 2-layer
MLP (768 -> relu 384 -> 8) over its expert's samples:

  layer 1:  h^T = relu(W1^T x^T + b1)   as matmul(psum, lhsT=W1 [128,128],
            rhs=xT [128,n]) accumulated over 6 d-blocks per h-block
  layer 2:  y^T = W2^T h^T  (+ b2 on host)

Matmul operands are bfloat16 (x and W quantized on host; PSUM accumulation
stays fp32): same 1-column/cycle PE stream rate as f32r but half the HBM
traffic for x.  Startup: all constant DMAs plus six x spans are queued
immediately and the lead spans are sized to the DMA ramp's supply curve, so
the PE never idles long enough for the HAM clock gate to re-throttle; a
continuous run of dummy matmuls bridges the PE from the preamble into the
first real matmul at 2.4 GHz.  Layer-2 matmul trios (one per h-block, on
three PE column groups so they stream concurrently) are batched per ~2048-col
group so the W1<->W2 weight-switch bubble is paid once per group.  The three
column-group partials are NOT folded on-chip: one DVE copy moves PSUM
partitions 0..71 to SBUF and the [72, n] slab is DMA'd out per chunk; the
host sums partials + b2 during unpacking (the fold on 8-partition operands
would cost 3 DVE ops per chunk and serializes into the kernel tail).  The
last span ends in a small remainder chunk so the final store's DMA receipt
chain is short.  Output partials [72, S] are folded/scattered on the host.
"""

import ml_dtypes
import numpy as np

import concourse.bass as bass
import concourse.mybir as mybir
from concourse.tile import TileContext
from concourse.bass_utils import run_bass_kernel_spmd

P = 128
D = 768
H = 384
C = 8
E = 8
NCORES = 8
DBLK = D // P  # 6
HBLK = H // P  # 3
CHUNK = 512  # compute chunk (one PSUM bank of fp32)
XGRAN = 1024  # steady-state x DMA granularity (samples per load)
YROWS = 64 + C  # partial slab height: col-group partials at 0-7/32-39/64-71
NPREF = 6  # x spans queued before the main loop starts
NWARM = 11  # dummy matmuls that bridge the HAM clock gate during startup DMA
WARMN = 256  # columns per warmup matmul

BF16 = ml_dtypes.bfloat16

_program_cache = {}
last_results = None  # BassKernelResults of the most recent run (for test harness)


def _split_excess_waits(nc, max_waits=1):
    """The walrus build in this container only encodes one sem-wait per
    instruction; hoist extra waits onto NOPs inserted just before."""
    for blk in nc.main_func.blocks:
        insts = blk.instructions
        i = 0
        while i < len(insts):
            inst = insts[i]
            si = getattr(inst, "sync_info", None)
            if si is not None and si.on_wait and len(si.on_wait) > max_waits:
                waits = list(si.on_wait)
                extra, keep = waits[:-max_waits], waits[-max_waits:]
                nops = []
                for j in range(0, len(extra), max_waits):
                    nops.append(
                        mybir.InstNoOp(
                            name=f"{inst.name}-wsplit{j}",
                            engine=inst.engine,
                            bass_nofuse=True,
                            sync_info=mybir.SyncInfo(
                                on_wait=extra[j : j + max_waits], on_update=[]
                            ),
                        )
                    )
                inst.sync_info = mybir.SyncInfo(on_wait=keep, on_update=si.on_update)
                for k, nop in enumerate(nops):
                    nc.register_instruction(nop, overwrite=True)
                    insts.insert(i + k, nop)
                i += len(nops)
            i += 1
    return nc


def _x_spans(S):
    """Span decomposition of S, shared by the program and the host packer.
    Small lead spans track the DMA ramp's supply curve so the warm PE never
    starves long enough to re-throttle; the bulk streams as XGRAN spans; the
    odd remainder goes LAST so the kernel tail ends on a small chunk (short
    epilogue + tiny final store)."""
    spans = []
    off = 0
    for n in (128, 128, 256, 256, 512, 512):
        n = min(n, S - off)
        if n <= 0:
            break
        spans.append((off, n))
        off += n
    while S - off > XGRAN:
        spans.append((off, XGRAN))
        off += XGRAN
    r = S - off
    if r > CHUNK:
        spans.append((off, CHUNK))
        off += CHUNK
        r -= CHUNK
    if r > 128:
        # split so the very last chunk is small: its relu->matmul->copy->
        # store chain is the kernel's final dependent path
        spans.append((off, r - 128))
        off += r - 128
        r = 128
    if r > 0:
        spans.append((off, r))
    return spans


def _build_program(S):
    f32 = mybir.dt.float32
    bf16 = mybir.dt.bfloat16
    relu = mybir.ActivationFunctionType.Relu

    nc = bass.Bass(enable_partition_id=False)
    # x is packed span-contiguous: for each span, [P, DBLK*n] with the six
    # d-blocks back to back, so every span loads as one maximal-row DMA
    # (sub-2KB per-partition rows run far below DMA line rate).
    xt = nc.dram_tensor("xt", [P, DBLK * S], bf16, kind="ExternalInput")
    # w1 (6*384 cols) and w2 (3*8 cols) packed on the same 128 partitions
    wt = nc.dram_tensor("wt", [P, DBLK * H + HBLK * C], bf16, kind="ExternalInput")
    # b1 (3 cols, per h-block); b2 is added on the host
    bt = nc.dram_tensor("bt", [P, HBLK], f32, kind="ExternalInput")
    yt = nc.dram_tensor("yt", [YROWS, S], f32, kind="ExternalOutput")

    x_spans = _x_spans(S)
    # chunk list: (span_idx, global_off, span_off, n)
    chunks = []
    for si, (soff, sn) in enumerate(x_spans):
        for o in range(0, sn, CHUNK):
            chunks.append((si, soff + o, o, min(CHUNK, sn - o)))
    # layer-2 flush groups: <= 4 chunks (one PSUM bank per chunk during the
    # batched flush).  The final group flushes per-chunk, pipelined.
    groups = []
    cur, cols = [], 0
    for ci, c in enumerate(chunks):
        if cur and (cols + c[3] > 2048 or len(cur) == 4):
            groups.append(cur)
            cur, cols = [], 0
        cur.append(ci)
        cols += c[3]
    groups.append(cur)
    # the chunk at whose hb0 point group g flushes: first chunk of group g+1
    flush_after = {}
    for g in range(len(groups) - 1):
        flush_after[groups[g + 1][0]] = g

    with TileContext(nc) as tc:
        with (
            tc.tile_pool(name="const", bufs=1) as cpool,
            tc.tile_pool(name="xin", bufs=NPREF + 2) as xpool,
            tc.tile_pool(name="hbuf", bufs=6) as hpool,
            tc.tile_pool(name="yout", bufs=2) as ypool,
            tc.tile_pool(name="psum1", bufs=4, space="PSUM") as pp1,
            tc.tile_pool(name="psum2", bufs=4, space="PSUM") as pp2,
        ):
            # HAM warmup operands (zeros; result never read).  Both memsets
            # on gpsimd: it clears its preamble ~1us before the DVE does,
            # so the warmup matmul stream starts that much earlier.
            warm_w = cpool.tile([P, P], bf16)
            nc.gpsimd.memset(warm_w[:], 0.0)
            warm_x = cpool.tile([P, WARMN], bf16)
            nc.gpsimd.memset(warm_x[:], 0.0)

            w_t = cpool.tile([P, DBLK * H + HBLK * C], bf16)
            b_t = cpool.tile([P, HBLK], f32)
            span_tiles = {}

            def load_x(span_idx, eng=None):
                off, n = x_spans[span_idx]
                x_t = xpool.tile([P, DBLK * XGRAN], bf16, name="x_t")
                if eng is None:
                    eng = nc.sync if span_idx % 2 == 0 else nc.scalar
                eng.dma_start(
                    x_t[:, : DBLK * n],
                    xt[:, DBLK * off : DBLK * (off + n)],
                )
                span_tiles[span_idx] = x_t

            # Startup DMA order: each queue delivers strictly in the order
            # the first chunks consume the data.  Tile's dependency tracking
            # is range-aware across multiple DMA writers of one tile, so
            # span 0 and w_hb0 arrive as ~100KB halves and chunk 0's first
            # matmuls gate on half the bytes (the DMA ramp makes the first
            # ~0.5MB cost ~3us; halving the first gate starts the PE ~1.5us
            # sooner).
            #   sync:   x0a, x0b, x1, b
            #   scalar: w0a, w0b, w_hb1, w_hb2, x2, x3, x4, x5
            h3 = 3 * P
            nc.scalar.dma_start(w_t[:, :h3], wt[:, :h3])
            s0n = x_spans[0][1]
            x_t0 = xpool.tile([P, DBLK * XGRAN], bf16, name="x_t")
            nc.sync.dma_start(x_t0[:, : 3 * s0n], xt[:, : 3 * s0n])
            nc.scalar.dma_start(w_t[:, h3 : DBLK * P], wt[:, h3 : DBLK * P])
            nc.sync.dma_start(
                x_t0[:, 3 * s0n : DBLK * s0n], xt[:, 3 * s0n : DBLK * s0n]
            )
            span_tiles[0] = x_t0
            nc.scalar.dma_start(
                w_t[:, DBLK * P : 2 * DBLK * P], wt[:, DBLK * P : 2 * DBLK * P]
            )
            load_x(1, nc.sync)
            nc.scalar.dma_start(w_t[:, 2 * DBLK * P :], wt[:, 2 * DBLK * P :])
            nc.sync.dma_start(b_t[:], bt[:])
            load_x(2, nc.scalar)
            load_x(3, nc.scalar)
            if len(x_spans) > 4:
                load_x(4, nc.scalar)
            if len(x_spans) > 5:
                load_x(5, nc.scalar)

            # Warm the ACT table during the startup DMA window so the
            # first real relu doesn't pay the ~1.5us table load.
            warm_a = cpool.tile([P, 1], f32)
            nc.any.memset(warm_a[:], 0.0)
            nc.scalar.activation(warm_a[:], warm_a[:], relu, bias=0.0)

            # Dummy matmuls: keep the PE continuously busy from the end of
            # the preamble until real data lands, so the HAM clock gate
            # opens (K=8/8) with no intervening idle window.
            warm_ps = pp1.tile([P, CHUNK], f32, name="ps")
            for _ in range(NWARM):
                nc.tensor.matmul(
                    warm_ps[:, :WARMN], warm_w[:], warm_x[:],
                    start=True, stop=True,
                )

            h_tiles = {}  # chunk idx -> (h_t, global_off, n)

            def emit_trio(ci):
                # layer 2 for an already-relu'd chunk: the three h-block
                # matmuls go to three PE column-group tiles (psum partitions
                # 0-7 / 32-39 / 64-71) so they stream concurrently.
                h_t, off, n = h_tiles.pop(ci)
                ps2 = pp2.tile([P, CHUNK], f32, name="ps2")
                for hb in range(HBLK):
                    nc.tensor.matmul(
                        ps2[32 * hb : 32 * hb + C, :n],
                        w_t[:, DBLK * H + hb * C : DBLK * H + (hb + 1) * C],
                        h_t[:, hb, :n],
                        start=True,
                        stop=True,
                        tile_position=(0, 32 * hb),
                    )
                return ps2, off, n

            def emit_store(ps2, off, n):
                # single wide PSUM->SBUF copy (partials stay unfolded; the
                # 8-partition fold ops would serialize on DVE), then DMA out.
                # Stores go on the sync queue ONLY: a store waiting on its
                # copy must never sit in front of relu ACTIVATEs (scalar).
                y_s = ypool.tile([YROWS, CHUNK], f32, name="y_s")
                nc.vector.tensor_copy(y_s[:, :n], ps2[:YROWS, :n])
                nc.sync.dma_start(yt[:, off : off + n], y_s[:, :n])

            def flush_group(g):
                # batched: all trios back to back (W2 stays stationary in
                # the three col-group tiles across the whole group), then
                # one wide copy per chunk into the group staging tile and a
                # single group store on the sync queue.
                trio_out = [emit_trio(ci) for ci in groups[g]]
                y_g = ypool.tile([YROWS, 2048], f32, name="y_g")
                base = trio_out[0][1]
                for ps2, off, n in trio_out:
                    nc.vector.tensor_copy(
                        y_g[:, off - base : off - base + n], ps2[:YROWS, :n]
                    )
                width = trio_out[-1][1] - base + trio_out[-1][2]
                nc.sync.dma_start(yt[:, base : base + width], y_g[:, :width])

            last_ci = len(chunks) - 1
            final_group = set(groups[-1])
            for ci, (si, goff, o, n) in enumerate(chunks):
                x_t = span_tiles[si]
                h_t = hpool.tile([P, HBLK, CHUNK], bf16, name="h_t")
                for hb in range(HBLK):
                    ps = pp1.tile([P, CHUNK], f32, name="ps")
                    for db in range(DBLK):
                        nc.tensor.matmul(
                            ps[:, :n],
                            w_t[:, (hb * DBLK + db) * P : (hb * DBLK + db + 1) * P],
                            x_t[:, db * x_spans[si][1] + o : db * x_spans[si][1] + o + n],
                            start=(db == 0),
                            stop=(db == DBLK - 1),
                        )
                    nc.scalar.activation(
                        h_t[:, hb, :n], ps[:, :n], relu,
                        bias=b_t[:, hb : hb + 1],
                    )
                    if hb == 0:
                        if o == 0 and si + NPREF < len(x_spans):
                            load_x(si + NPREF)
                        g = flush_after.get(ci)
                        if g is not None:
                            flush_group(g)
                        if ci in final_group and ci - 1 in h_tiles:
                            # tail: previous chunk's layer 2, pipelined
                            ps2, poff, pn = emit_trio(ci - 1)
                            emit_store(ps2, poff, pn)
                h_tiles[ci] = (h_t, goff, n)

            # final chunk: PSUM-accumulate layer 2 (all three h-blocks fold
            # into partitions 0-7 in PSUM), so the epilogue is a tiny [C, n]
            # copy + store and the final DMA-receipt chain is short.  The
            # host reads only rows 0-7 for these columns.
            h_t, off, n = h_tiles.pop(last_ci)
            ps2 = pp2.tile([P, CHUNK], f32, name="ps2")
            for hb in range(HBLK):
                nc.tensor.matmul(
                    ps2[:C, :n],
                    w_t[:, DBLK * H + hb * C : DBLK * H + (hb + 1) * C],
                    h_t[:, hb, :n],
                    start=(hb == 0),
                    stop=(hb == HBLK - 1),
                    tile_position=(0, 0),
                )
            y_f = ypool.tile([C, CHUNK], f32, name="y_f")
            nc.vector.tensor_copy(y_f[:, :n], ps2[:C, :n])
            nc.sync.dma_start(yt[:C, off : off + n], y_f[:, :n])

    return _split_excess_waits(nc)


def kernel(x, W1, b1, W2, b2, question_types):
    global last_results
    x = np.ascontiguousarray(np.asarray(x, dtype=np.float32))
    W1 = np.asarray(W1, dtype=np.float32)
    b1 = np.asarray(b1, dtype=np.float32)
    W2 = np.asarray(W2, dtype=np.float32)
    b2 = np.asarray(b2, dtype=np.float32)
    qt = np.asarray(question_types)
    N = x.shape[0]

    idx = [np.nonzero(qt == e)[0] for e in range(E)]
    counts = [len(i) for i in idx]
    S = max(int(np.ceil(max(counts) / 16) * 16), 4 * CHUNK)

    nc = _program_cache.get(S)
    if nc is None:
        nc = _build_program(S)
        _program_cache[S] = nc

    x16 = x.astype(BF16)
    spans = _x_spans(S)
    in_maps = []
    for e in range(E):
        cnt = counts[e]
        xt_f = np.zeros((P, DBLK, S), BF16)
        xt_f[:, :, :cnt] = x16[idx[e]].reshape(cnt, DBLK, P).transpose(2, 1, 0)
        # span-contiguous repack: [P, sum_over_spans(DBLK*n)]
        xt = np.concatenate(
            [xt_f[:, :, off : off + n].reshape(P, DBLK * n) for off, n in spans],
            axis=1,
        )
        # hb-major layout so the first 768 cols (= hb0's six d-blocks) can
        # be DMA'd first and gate only the first chunk's matmuls
        w1t = (
            W1[e]
            .reshape(DBLK, P, HBLK, P)
            .transpose(1, 2, 0, 3)
            .reshape(P, DBLK * H)
        )
        w2t = W2[e].reshape(HBLK, P, C).transpose(1, 0, 2).reshape(P, HBLK * C)
        wt = np.concatenate([w1t, w2t], axis=1).astype(BF16)
        bt = b1[e].reshape(HBLK, P).T.copy()
        in_maps.append({"xt": xt, "wt": wt, "bt": bt})

    r = run_bass_kernel_spmd(nc, in_maps, list(range(NCORES)))
    last_results = r

    # the final chunk's columns hold the already-folded result in rows 0-7
    last_soff, last_sn = spans[-1]
    fin = last_soff + ((last_sn - 1) // CHUNK) * CHUNK
    out = np.zeros((N, C), np.float32)
    for e in range(E):
        p = r.results[e]["yt"]
        y = p[0:C] + p[32 : 32 + C] + p[64 : 64 + C]
        y[:, fin:] = p[0:C, fin:]
        y += b2[e][:, None]
        out[idx[e]] = y[:, : counts[e]].T
    return out


# revision 33
# speedup vs baseline: 1.0016x; 1.0016x over previous
"""Routed MoE classifier head for Trainium2 (8 NeuronCores, SPMD).

The reference computes all 8 experts densely and selects; here each sample is
routed to exactly one expert.  On the host we gather samples by expert
(expert e -> core e), pad to a common S, and pre-transpose x so the
contraction dim D lands on SBUF partitions.  Each core runs a dense 2-layer
MLP (768 -> relu 384 -> 8) over its expert's samples:

  layer 1:  h^T = relu(W1^T x^T + b1)   as matmul(psum, lhsT=W1 [128,128],
            rhs=xT [128,n]) accumulated over 6 d-blocks per h-block
  layer 2:  y^T = W2^T h^T  (+ b2 on host)

Matmul operands are bfloat16 (x and W quantized on host; PSUM accumulation
stays fp32): same 1-column/cycle PE stream rate as f32r but half the HBM
traffic for x.  Startup: all constant DMAs plus six x spans are queued
immediately and the lead spans are sized to the DMA ramp's supply curve, so
the PE never idles long enough for the HAM clock gate to re-throttle; a
continuous run of dummy matmuls bridges the PE from the preamble into the
first real matmul at 2.4 GHz.  Layer-2 matmul trios (one per h-block, on
three PE column groups so they stream concurrently) are batched per ~2048-col
group so the W1<->W2 weight-switch bubble is paid once per group.  The three
column-group partials are NOT folded on-chip: one DVE copy moves PSUM
partitions 0..71 to SBUF and the [72, n] slab is DMA'd out per chunk; the
host sums partials + b2 during unpacking (the fold on 8-partition operands
would cost 3 DVE ops per chunk and serializes into the kernel tail).  The
last span ends in a small remainder chunk so the final store's DMA receipt
chain is short.  Output partials [72, S] are folded/scattered on the host.
"""

import ml_dtypes
import numpy as np

import concourse.bass as bass
import concourse.mybir as mybir
from concourse.tile import TileContext
from concourse.bass_utils import run_bass_kernel_spmd

P = 128
D = 768
H = 384
C = 8
E = 8
NCORES = 8
DBLK = D // P  # 6
HBLK = H // P  # 3
CHUNK = 512  # compute chunk (one PSUM bank of fp32)
XGRAN = 1024  # steady-state x DMA granularity (samples per load)
YROWS = 64 + C  # partial slab height: col-group partials at 0-7/32-39/64-71
NPREF = 6  # x spans queued before the main loop starts
NWARM = 18  # dummy matmuls that bridge the HAM clock gate during startup DMA
WARMN = 256  # columns per warmup matmul

BF16 = ml_dtypes.bfloat16

_program_cache = {}
last_results = None  # BassKernelResults of the most recent run (for test harness)


def _split_excess_waits(nc, max_waits=1):
    """The walrus build in this container only encodes one sem-wait per
    instruction; hoist extra waits onto NOPs inserted just before."""
    for blk in nc.main_func.blocks:
        insts = blk.instructions
        i = 0
        while i < len(insts):
            inst = insts[i]
            si = getattr(inst, "sync_info", None)
            if si is not None and si.on_wait and len(si.on_wait) > max_waits:
                waits = list(si.on_wait)
                extra, keep = waits[:-max_waits], waits[-max_waits:]
                nops = []
                for j in range(0, len(extra), max_waits):
                    nops.append(
                        mybir.InstNoOp(
                            name=f"{inst.name}-wsplit{j}",
                            engine=inst.engine,
                            bass_nofuse=True,
                            sync_info=mybir.SyncInfo(
                                on_wait=extra[j : j + max_waits], on_update=[]
                            ),
                        )
                    )
                inst.sync_info = mybir.SyncInfo(on_wait=keep, on_update=si.on_update)
                for k, nop in enumerate(nops):
                    nc.register_instruction(nop, overwrite=True)
                    insts.insert(i + k, nop)
                i += len(nops)
            i += 1
    return nc


def _x_spans(S):
    """Span decomposition of S, shared by the program and the host packer.
    Small lead spans track the DMA ramp's supply curve so the warm PE never
    starves long enough to re-throttle; the bulk streams as XGRAN spans; the
    odd remainder goes LAST so the kernel tail ends on a small chunk (short
    epilogue + tiny final store)."""
    spans = []
    off = 0
    for n in (128, 128, 256, 256, 512, 512):
        n = min(n, S - off)
        if n <= 0:
            break
        spans.append((off, n))
        off += n
    while S - off > XGRAN:
        spans.append((off, XGRAN))
        off += XGRAN
    r = S - off
    if r > CHUNK:
        spans.append((off, CHUNK))
        off += CHUNK
        r -= CHUNK
    if r > 128:
        # split so the very last chunk is small: its relu->matmul->copy->
        # store chain is the kernel's final dependent path
        spans.append((off, r - 128))
        off += r - 128
        r = 128
    if r > 0:
        spans.append((off, r))
    return spans


def _build_program(S):
    f32 = mybir.dt.float32
    bf16 = mybir.dt.bfloat16
    relu = mybir.ActivationFunctionType.Relu

    nc = bass.Bass(enable_partition_id=False)
    # x is packed span-contiguous: for each span, [P, DBLK*n] with the six
    # d-blocks back to back, so every span loads as one maximal-row DMA
    # (sub-2KB per-partition rows run far below DMA line rate).
    xt = nc.dram_tensor("xt", [P, DBLK * S], bf16, kind="ExternalInput")
    # w1 (6*384 cols) and w2 (3*8 cols) packed on the same 128 partitions
    wt = nc.dram_tensor("wt", [P, DBLK * H + HBLK * C], bf16, kind="ExternalInput")
    # b1 (3 cols, per h-block); b2 is added on the host
    bt = nc.dram_tensor("bt", [P, HBLK], f32, kind="ExternalInput")
    yt = nc.dram_tensor("yt", [YROWS, S], f32, kind="ExternalOutput")

    x_spans = _x_spans(S)
    # chunk list: (span_idx, global_off, span_off, n)
    chunks = []
    for si, (soff, sn) in enumerate(x_spans):
        for o in range(0, sn, CHUNK):
            chunks.append((si, soff + o, o, min(CHUNK, sn - o)))
    # layer-2 flush groups: <= 4 chunks (one PSUM bank per chunk during the
    # batched flush).  The final group flushes per-chunk, pipelined.
    groups = []
    cur, cols = [], 0
    for ci, c in enumerate(chunks):
        if cur and (cols + c[3] > 2048 or len(cur) == 4):
            groups.append(cur)
            cur, cols = [], 0
        cur.append(ci)
        cols += c[3]
    groups.append(cur)
    # the chunk at whose hb0 point group g flushes: first chunk of group g+1
    flush_after = {}
    for g in range(len(groups) - 1):
        flush_after[groups[g + 1][0]] = g

    with TileContext(nc) as tc:
        with (
            tc.tile_pool(name="const", bufs=1) as cpool,
            tc.tile_pool(name="xin", bufs=NPREF + 2) as xpool,
            tc.tile_pool(name="hbuf", bufs=6) as hpool,
            tc.tile_pool(name="yout", bufs=2) as ypool,
            tc.tile_pool(name="psum1", bufs=4, space="PSUM") as pp1,
            tc.tile_pool(name="psum2", bufs=4, space="PSUM") as pp2,
        ):
            # HAM warmup operands (zeros; result never read).  Both memsets
            # on gpsimd: it clears its preamble ~1us before the DVE does,
            # so the warmup matmul stream starts that much earlier.
            warm_w = cpool.tile([P, P], bf16)
            nc.gpsimd.memset(warm_w[:], 0.0)
            warm_x = cpool.tile([P, WARMN], bf16)
            nc.gpsimd.memset(warm_x[:], 0.0)

            w_t = cpool.tile([P, DBLK * H + HBLK * C], bf16)
            b_t = cpool.tile([P, HBLK], f32)
            span_tiles = {}

            def load_x(span_idx, eng=None):
                off, n = x_spans[span_idx]
                x_t = xpool.tile([P, DBLK * XGRAN], bf16, name="x_t")
                if eng is None:
                    eng = nc.sync if span_idx % 2 == 0 else nc.scalar
                eng.dma_start(
                    x_t[:, : DBLK * n],
                    xt[:, DBLK * off : DBLK * (off + n)],
                )
                span_tiles[span_idx] = x_t

            # Startup DMA order: each queue delivers strictly in the order
            # the first chunks consume the data, and the early-critical
            # pieces (x0 and x1, which gate chunks 0/1 during the DMA ramp)
            # sit at the head of a queue of their own.  (Sub-span gating of
            # x0/w_hb0 was tried: it starts the PE ~1.3us sooner but the
            # ramp then stalls chunk 0 mid-stream, which delays the HAM
            # un-throttle by ~6us -- a continuous late start wins.)
            #   sync:   x0, x1, b
            #   scalar: w_hb0, w_hb1, w_hb2, x2, x3, x4, x5
            nc.scalar.dma_start(w_t[:, : DBLK * P], wt[:, : DBLK * P])
            load_x(0, nc.sync)
            nc.scalar.dma_start(
                w_t[:, DBLK * P : 2 * DBLK * P], wt[:, DBLK * P : 2 * DBLK * P]
            )
            load_x(1, nc.sync)
            nc.scalar.dma_start(w_t[:, 2 * DBLK * P :], wt[:, 2 * DBLK * P :])
            nc.sync.dma_start(b_t[:], bt[:])
            load_x(2, nc.scalar)
            load_x(3, nc.scalar)
            if len(x_spans) > 4:
                load_x(4, nc.scalar)
            if len(x_spans) > 5:
                load_x(5, nc.scalar)

            # Warm the ACT table during the startup DMA window so the
            # first real relu doesn't pay the ~1.5us table load.
            warm_a = cpool.tile([P, 1], f32)
            nc.any.memset(warm_a[:], 0.0)
            nc.scalar.activation(warm_a[:], warm_a[:], relu, bias=0.0)

            # Dummy matmuls: keep the PE continuously busy from the end of
            # the preamble until real data lands, so the HAM clock gate
            # opens (K=8/8) with no intervening idle window.
            warm_ps = pp1.tile([P, CHUNK], f32, name="ps")
            for _ in range(NWARM):
                nc.tensor.matmul(
                    warm_ps[:, :WARMN], warm_w[:], warm_x[:],
                    start=True, stop=True,
                )

            h_tiles = {}  # chunk idx -> (h_t, global_off, n)

            def emit_trio(ci):
                # layer 2 for an already-relu'd chunk: the three h-block
                # matmuls go to three PE column-group tiles (psum partitions
                # 0-7 / 32-39 / 64-71) so they stream concurrently.
                h_t, off, n = h_tiles.pop(ci)
                ps2 = pp2.tile([P, CHUNK], f32, name="ps2")
                for hb in range(HBLK):
                    nc.tensor.matmul(
                        ps2[32 * hb : 32 * hb + C, :n],
                        w_t[:, DBLK * H + hb * C : DBLK * H + (hb + 1) * C],
                        h_t[:, hb, :n],
                        start=True,
                        stop=True,
                        tile_position=(0, 32 * hb),
                    )
                return ps2, off, n

            def emit_store(ps2, off, n):
                # single wide PSUM->SBUF copy (partials stay unfolded; the
                # 8-partition fold ops would serialize on DVE), then DMA out.
                # Stores go on the sync queue ONLY: a store waiting on its
                # copy must never sit in front of relu ACTIVATEs (scalar).
                y_s = ypool.tile([YROWS, CHUNK], f32, name="y_s")
                nc.vector.tensor_copy(y_s[:, :n], ps2[:YROWS, :n])
                nc.sync.dma_start(yt[:, off : off + n], y_s[:, :n])

            def flush_group(g):
                # batched: all trios back to back (W2 stays stationary in
                # the three col-group tiles across the whole group), then
                # one wide copy per chunk into the group staging tile and a
                # single group store on the sync queue.
                trio_out = [emit_trio(ci) for ci in groups[g]]
                y_g = ypool.tile([YROWS, 2048], f32, name="y_g")
                base = trio_out[0][1]
                for ps2, off, n in trio_out:
                    nc.vector.tensor_copy(
                        y_g[:, off - base : off - base + n], ps2[:YROWS, :n]
                    )
                width = trio_out[-1][1] - base + trio_out[-1][2]
                nc.sync.dma_start(yt[:, base : base + width], y_g[:, :width])

            last_ci = len(chunks) - 1
            final_group = set(groups[-1])
            for ci, (si, goff, o, n) in enumerate(chunks):
                x_t = span_tiles[si]
                h_t = hpool.tile([P, HBLK, CHUNK], bf16, name="h_t")
                for hb in range(HBLK):
                    ps = pp1.tile([P, CHUNK], f32, name="ps")
                    for db in range(DBLK):
                        nc.tensor.matmul(
                            ps[:, :n],
                            w_t[:, (hb * DBLK + db) * P : (hb * DBLK + db + 1) * P],
                            x_t[:, db * x_spans[si][1] + o : db * x_spans[si][1] + o + n],
                            start=(db == 0),
                            stop=(db == DBLK - 1),
                        )
                    nc.scalar.activation(
                        h_t[:, hb, :n], ps[:, :n], relu,
                        bias=b_t[:, hb : hb + 1],
                    )
                    if hb == 0:
                        if o == 0 and si + NPREF < len(x_spans):
                            load_x(si + NPREF)
                        g = flush_after.get(ci)
                        if g is not None:
                            flush_group(g)
                        if ci in final_group and ci - 1 in h_tiles:
                            # tail: previous chunk's layer 2, pipelined
                            ps2, poff, pn = emit_trio(ci - 1)
                            emit_store(ps2, poff, pn)
                h_tiles[ci] = (h_t, goff, n)

            # final chunk: PSUM-accumulate layer 2 (all three h-blocks fold
            # into partitions 0-7 in PSUM), so the epilogue is a tiny [C, n]
            # copy + store and the final DMA-receipt chain is short.  The
            # host reads only rows 0-7 for these columns.
            h_t, off, n = h_tiles.pop(last_ci)
            ps2 = pp2.tile([P, CHUNK], f32, name="ps2")
            for hb in range(HBLK):
                nc.tensor.matmul(
                    ps2[:C, :n],
                    w_t[:, DBLK * H + hb * C : DBLK * H + (hb + 1) * C],
                    h_t[:, hb, :n],
                    start=(hb == 0),
                    stop=(hb == HBLK - 1),
                    tile_position=(0, 0),
                )
            y_f = ypool.tile([C, CHUNK], f32, name="y_f")
            nc.vector.tensor_copy(y_f[:, :n], ps2[:C, :n])
            nc.sync.dma_start(yt[:C, off : off + n], y_f[:, :n])

    return _split_excess_waits(nc)


def kernel(x, W1, b1, W2, b2, question_types):
    global last_results
    x = np.ascontiguousarray(np.asarray(x, dtype=np.float32))
    W1 = np.asarray(W1, dtype=np.float32)
    b1 = np.asarray(b1, dtype=np.float32)
    W2 = np.asarray(W2, dtype=np.float32)
    b2 = np.asarray(b2, dtype=np.float32)
    qt = np.asarray(question_types)
    N = x.shape[0]

    idx = [np.nonzero(qt == e)[0] for e in range(E)]
    counts = [len(i) for i in idx]
    S = max(int(np.ceil(max(counts) / 16) * 16), 4 * CHUNK)

    nc = _program_cache.get(S)
    if nc is None:
        nc = _build_program(S)
        _program_cache[S] = nc

    x16 = x.astype(BF16)
    spans = _x_spans(S)
    in_maps = []
    for e in range(E):
        cnt = counts[e]
        xt_f = np.zeros((P, DBLK, S), BF16)
        xt_f[:, :, :cnt] = x16[idx[e]].reshape(cnt, DBLK, P).transpose(2, 1, 0)
        # span-contiguous repack: [P, sum_over_spans(DBLK*n)]
        xt = np.concatenate(
            [xt_f[:, :, off : off + n].reshape(P, DBLK * n) for off, n in spans],
            axis=1,
        )
        # hb-major layout so the first 768 cols (= hb0's six d-blocks) can
        # be DMA'd first and gate only the first chunk's matmuls
        w1t = (
            W1[e]
            .reshape(DBLK, P, HBLK, P)
            .transpose(1, 2, 0, 3)
            .reshape(P, DBLK * H)
        )
        w2t = W2[e].reshape(HBLK, P, C).transpose(1, 0, 2).reshape(P, HBLK * C)
        wt = np.concatenate([w1t, w2t], axis=1).astype(BF16)
        bt = b1[e].reshape(HBLK, P).T.copy()
        in_maps.append({"xt": xt, "wt": wt, "bt": bt})

    r = run_bass_kernel_spmd(nc, in_maps, list(range(NCORES)))
    last_results = r

    # the final chunk's columns hold the already-folded result in rows 0-7
    last_soff, last_sn = spans[-1]
    fin = last_soff + ((last_sn - 1) // CHUNK) * CHUNK
    out = np.zeros((N, C), np.float32)
    for e in range(E):
        p = r.results[e]["yt"]
        y = p[0:C] + p[32 : 32 + C] + p[64 : 64 + C]
        y[:, fin:] = p[0:C, fin:]
        y += b2[e][:, None]
        out[idx[e]] = y[:, : counts[e]].T
    return out


# revision 35
# speedup vs baseline: 1.0057x; 1.0040x over previous
"""Routed MoE classifier head for Trainium2 (8 NeuronCores, SPMD).

The reference computes all 8 experts densely and selects; here each sample is
routed to exactly one expert.  On the host we gather samples by expert
(expert e -> core e), pad to a common S, and pre-transpose x so the
contraction dim D lands on SBUF partitions.  Each core runs a dense 2-layer
MLP (768 -> relu 384 -> 8) over its expert's samples:

  layer 1:  h^T = relu(W1^T x^T + b1)   as matmul(psum, lhsT=W1 [128,128],
            rhs=xT [128,n]) accumulated over 6 d-blocks per h-block
  layer 2:  y^T = W2^T h^T  (+ b2 on host)

Matmul operands are bfloat16 (x and W quantized on host; PSUM accumulation
stays fp32): same 1-column/cycle PE stream rate as f32r but half the HBM
traffic for x.  Startup: all constant DMAs plus six x spans are queued
immediately and the lead spans are sized to the DMA ramp's supply curve, so
the PE never idles long enough for the HAM clock gate to re-throttle; a
continuous run of dummy matmuls bridges the PE from the preamble into the
first real matmul at 2.4 GHz.  Layer-2 matmul trios (one per h-block, on
three PE column groups so they stream concurrently) are batched per ~2048-col
group so the W1<->W2 weight-switch bubble is paid once per group.  The three
column-group partials are NOT folded on-chip: one DVE copy moves PSUM
partitions 0..71 to SBUF and the [72, n] slab is DMA'd out per chunk; the
host sums partials + b2 during unpacking (the fold on 8-partition operands
would cost 3 DVE ops per chunk and serializes into the kernel tail).  The
last span ends in a small remainder chunk so the final store's DMA receipt
chain is short.  Output partials [72, S] are folded/scattered on the host.
"""

import ml_dtypes
import numpy as np

import concourse.bass as bass
import concourse.mybir as mybir
from concourse.tile import TileContext
from concourse.bass_utils import run_bass_kernel_spmd

P = 128
D = 768
H = 384
C = 8
E = 8
NCORES = 8
DBLK = D // P  # 6
HBLK = H // P  # 3
CHUNK = 512  # compute chunk (one PSUM bank of fp32)
XGRAN = 1024  # steady-state x DMA granularity (samples per load)
YROWS = 64 + C  # partial slab height: col-group partials at 0-7/32-39/64-71
NPREF = 6  # x spans queued before the main loop starts
NWARM = 18  # dummy matmuls that bridge the HAM clock gate during startup DMA
WARMN = 256  # columns per warmup matmul

BF16 = ml_dtypes.bfloat16

_program_cache = {}
last_results = None  # BassKernelResults of the most recent run (for test harness)


def _split_excess_waits(nc, max_waits=1):
    """The walrus build in this container only encodes one sem-wait per
    instruction; hoist extra waits onto NOPs inserted just before."""
    for blk in nc.main_func.blocks:
        insts = blk.instructions
        i = 0
        while i < len(insts):
            inst = insts[i]
            si = getattr(inst, "sync_info", None)
            if si is not None and si.on_wait and len(si.on_wait) > max_waits:
                waits = list(si.on_wait)
                extra, keep = waits[:-max_waits], waits[-max_waits:]
                nops = []
                for j in range(0, len(extra), max_waits):
                    nops.append(
                        mybir.InstNoOp(
                            name=f"{inst.name}-wsplit{j}",
                            engine=inst.engine,
                            bass_nofuse=True,
                            sync_info=mybir.SyncInfo(
                                on_wait=extra[j : j + max_waits], on_update=[]
                            ),
                        )
                    )
                inst.sync_info = mybir.SyncInfo(on_wait=keep, on_update=si.on_update)
                for k, nop in enumerate(nops):
                    nc.register_instruction(nop, overwrite=True)
                    insts.insert(i + k, nop)
                i += len(nops)
            i += 1
    return nc


def _x_spans(S):
    """Span decomposition of S, shared by the program and the host packer.
    Small lead spans track the DMA ramp's supply curve so the warm PE never
    starves long enough to re-throttle; the bulk streams as XGRAN spans; the
    odd remainder goes LAST so the kernel tail ends on a small chunk (short
    epilogue + tiny final store)."""
    spans = []
    off = 0
    for n in (128, 128, 256, 256, 512, 512):
        n = min(n, S - off)
        if n <= 0:
            break
        spans.append((off, n))
        off += n
    while S - off > XGRAN:
        spans.append((off, XGRAN))
        off += XGRAN
    r = S - off
    if r > CHUNK:
        spans.append((off, CHUNK))
        off += CHUNK
        r -= CHUNK
    if r > 64:
        # split so the very last chunk is tiny: its relu->matmul->copy->
        # store chain is the kernel's final dependent path
        spans.append((off, r - 64))
        off += r - 64
        r = 64
    if r > 0:
        spans.append((off, r))
    return spans


def _build_program(S):
    f32 = mybir.dt.float32
    bf16 = mybir.dt.bfloat16
    relu = mybir.ActivationFunctionType.Relu

    nc = bass.Bass(enable_partition_id=False)
    # x is packed span-contiguous: for each span, [P, DBLK*n] with the six
    # d-blocks back to back, so every span loads as one maximal-row DMA
    # (sub-2KB per-partition rows run far below DMA line rate).
    xt = nc.dram_tensor("xt", [P, DBLK * S], bf16, kind="ExternalInput")
    # w1 (6*384 cols) and w2 (3*8 cols) packed on the same 128 partitions
    wt = nc.dram_tensor("wt", [P, DBLK * H + HBLK * C], bf16, kind="ExternalInput")
    # b1 (3 cols, per h-block); b2 is added on the host
    bt = nc.dram_tensor("bt", [P, HBLK], f32, kind="ExternalInput")
    yt = nc.dram_tensor("yt", [YROWS, S], f32, kind="ExternalOutput")

    x_spans = _x_spans(S)
    # chunk list: (span_idx, global_off, span_off, n)
    chunks = []
    for si, (soff, sn) in enumerate(x_spans):
        for o in range(0, sn, CHUNK):
            chunks.append((si, soff + o, o, min(CHUNK, sn - o)))
    # layer-2 flush groups: <= 4 chunks (one PSUM bank per chunk during the
    # batched flush).  The final group flushes per-chunk, pipelined.
    groups = []
    cur, cols = [], 0
    for ci, c in enumerate(chunks):
        if cur and (cols + c[3] > 2048 or len(cur) == 4):
            groups.append(cur)
            cur, cols = [], 0
        cur.append(ci)
        cols += c[3]
    groups.append(cur)
    # the chunk at whose hb0 point group g flushes: first chunk of group g+1
    flush_after = {}
    for g in range(len(groups) - 1):
        flush_after[groups[g + 1][0]] = g

    with TileContext(nc) as tc:
        with (
            tc.tile_pool(name="const", bufs=1) as cpool,
            tc.tile_pool(name="xin", bufs=NPREF + 2) as xpool,
            tc.tile_pool(name="hbuf", bufs=6) as hpool,
            tc.tile_pool(name="yout", bufs=2) as ypool,
            tc.tile_pool(name="psum1", bufs=4, space="PSUM") as pp1,
            tc.tile_pool(name="psum2", bufs=4, space="PSUM") as pp2,
        ):
            # HAM warmup operands (zeros; result never read).  Both memsets
            # on gpsimd: it clears its preamble ~1us before the DVE does,
            # so the warmup matmul stream starts that much earlier.
            warm_w = cpool.tile([P, P], bf16)
            nc.gpsimd.memset(warm_w[:], 0.0)
            warm_x = cpool.tile([P, WARMN], bf16)
            nc.gpsimd.memset(warm_x[:], 0.0)

            w_t = cpool.tile([P, DBLK * H + HBLK * C], bf16)
            b_t = cpool.tile([P, HBLK], f32)
            span_tiles = {}

            def load_x(span_idx, eng=None):
                off, n = x_spans[span_idx]
                x_t = xpool.tile([P, DBLK * XGRAN], bf16, name="x_t")
                if eng is None:
                    eng = nc.sync if span_idx % 2 == 0 else nc.scalar
                eng.dma_start(
                    x_t[:, : DBLK * n],
                    xt[:, DBLK * off : DBLK * (off + n)],
                )
                span_tiles[span_idx] = x_t

            # Startup DMA order: each queue delivers strictly in the order
            # the first chunks consume the data, and the early-critical
            # pieces (x0 and x1, which gate chunks 0/1 during the DMA ramp)
            # sit at the head of a queue of their own.  (Sub-span gating of
            # x0/w_hb0 was tried: it starts the PE ~1.3us sooner but the
            # ramp then stalls chunk 0 mid-stream, which delays the HAM
            # un-throttle by ~6us -- a continuous late start wins.)
            #   sync:   x0, x1, b
            #   scalar: w_hb0, w_hb1, w_hb2, x2, x3, x4, x5
            nc.scalar.dma_start(w_t[:, : DBLK * P], wt[:, : DBLK * P])
            load_x(0, nc.sync)
            nc.scalar.dma_start(
                w_t[:, DBLK * P : 2 * DBLK * P], wt[:, DBLK * P : 2 * DBLK * P]
            )
            load_x(1, nc.sync)
            nc.scalar.dma_start(w_t[:, 2 * DBLK * P :], wt[:, 2 * DBLK * P :])
            nc.sync.dma_start(b_t[:], bt[:])
            load_x(2, nc.scalar)
            load_x(3, nc.scalar)
            if len(x_spans) > 4:
                load_x(4, nc.scalar)
            if len(x_spans) > 5:
                load_x(5, nc.scalar)

            # Warm the ACT table during the startup DMA window so the
            # first real relu doesn't pay the ~1.5us table load.
            warm_a = cpool.tile([P, 1], f32)
            nc.any.memset(warm_a[:], 0.0)
            nc.scalar.activation(warm_a[:], warm_a[:], relu, bias=0.0)

            # Dummy matmuls: keep the PE continuously busy from the end of
            # the preamble until real data lands, so the HAM clock gate
            # opens (K=8/8) with no intervening idle window.
            warm_ps = pp1.tile([P, CHUNK], f32, name="ps")
            for _ in range(NWARM):
                nc.tensor.matmul(
                    warm_ps[:, :WARMN], warm_w[:], warm_x[:],
                    start=True, stop=True,
                )

            h_tiles = {}  # chunk idx -> (h_t, global_off, n)

            def emit_trio(ci):
                # layer 2 for an already-relu'd chunk: the three h-block
                # matmuls go to three PE column-group tiles (psum partitions
                # 0-7 / 32-39 / 64-71) so they stream concurrently.
                h_t, off, n = h_tiles.pop(ci)
                ps2 = pp2.tile([P, CHUNK], f32, name="ps2")
                for hb in range(HBLK):
                    nc.tensor.matmul(
                        ps2[32 * hb : 32 * hb + C, :n],
                        w_t[:, DBLK * H + hb * C : DBLK * H + (hb + 1) * C],
                        h_t[:, hb, :n],
                        start=True,
                        stop=True,
                        tile_position=(0, 32 * hb),
                    )
                return ps2, off, n

            def emit_store(ps2, off, n):
                # single wide PSUM->SBUF copy (partials stay unfolded; the
                # 8-partition fold ops would serialize on DVE), then DMA out.
                # Stores go on the sync queue ONLY: a store waiting on its
                # copy must never sit in front of relu ACTIVATEs (scalar).
                y_s = ypool.tile([YROWS, CHUNK], f32, name="y_s")
                nc.vector.tensor_copy(y_s[:, :n], ps2[:YROWS, :n])
                nc.sync.dma_start(yt[:, off : off + n], y_s[:, :n])

            def flush_group(g):
                # batched: all trios back to back (W2 stays stationary in
                # the three col-group tiles across the whole group), then
                # one wide copy per chunk into the group staging tile and a
                # single group store on the sync queue.
                trio_out = [emit_trio(ci) for ci in groups[g]]
                y_g = ypool.tile([YROWS, 2048], f32, name="y_g")
                base = trio_out[0][1]
                for ps2, off, n in trio_out:
                    nc.vector.tensor_copy(
                        y_g[:, off - base : off - base + n], ps2[:YROWS, :n]
                    )
                width = trio_out[-1][1] - base + trio_out[-1][2]
                nc.sync.dma_start(yt[:, base : base + width], y_g[:, :width])

            last_ci = len(chunks) - 1
            final_group = set(groups[-1])
            for ci, (si, goff, o, n) in enumerate(chunks):
                x_t = span_tiles[si]
                h_t = hpool.tile([P, HBLK, CHUNK], bf16, name="h_t")
                for hb in range(HBLK):
                    ps = pp1.tile([P, CHUNK], f32, name="ps")
                    for db in range(DBLK):
                        nc.tensor.matmul(
                            ps[:, :n],
                            w_t[:, (hb * DBLK + db) * P : (hb * DBLK + db + 1) * P],
                            x_t[:, db * x_spans[si][1] + o : db * x_spans[si][1] + o + n],
                            start=(db == 0),
                            stop=(db == DBLK - 1),
                        )
                    if ci == last_ci and hb == HBLK - 1:
                        # final chunk's gating relu on the DVE: shorter op,
                        # and it leads straight into the DVE copy
                        nc.vector.tensor_scalar(
                            h_t[:, hb, :n], ps[:, :n],
                            scalar1=b_t[:, hb : hb + 1], scalar2=0.0,
                            op0=mybir.AluOpType.add, op1=mybir.AluOpType.max,
                        )
                    else:
                        nc.scalar.activation(
                            h_t[:, hb, :n], ps[:, :n], relu,
                            bias=b_t[:, hb : hb + 1],
                        )
                    if hb == 0:
                        if o == 0 and si + NPREF < len(x_spans):
                            load_x(si + NPREF)
                        g = flush_after.get(ci)
                        if g is not None:
                            flush_group(g)
                        if ci in final_group and ci - 1 in h_tiles:
                            # tail: previous chunk's layer 2, pipelined
                            ps2, poff, pn = emit_trio(ci - 1)
                            emit_store(ps2, poff, pn)
                h_tiles[ci] = (h_t, goff, n)

            # final chunk: PSUM-accumulate layer 2 (all three h-blocks fold
            # into partitions 0-7 in PSUM), so the epilogue is a tiny [C, n]
            # copy + store and the final DMA-receipt chain is short.  The
            # host reads only rows 0-7 for these columns.
            h_t, off, n = h_tiles.pop(last_ci)
            ps2 = pp2.tile([P, CHUNK], f32, name="ps2")
            for hb in range(HBLK):
                nc.tensor.matmul(
                    ps2[:C, :n],
                    w_t[:, DBLK * H + hb * C : DBLK * H + (hb + 1) * C],
                    h_t[:, hb, :n],
                    start=(hb == 0),
                    stop=(hb == HBLK - 1),
                    tile_position=(0, 0),
                )
            y_f = ypool.tile([C, CHUNK], f32, name="y_f")
            nc.vector.tensor_copy(y_f[:, :n], ps2[:C, :n])
            nc.sync.dma_start(yt[:C, off : off + n], y_f[:, :n])

    return _split_excess_waits(nc)


def kernel(x, W1, b1, W2, b2, question_types):
    global last_results
    x = np.ascontiguousarray(np.asarray(x, dtype=np.float32))
    W1 = np.asarray(W1, dtype=np.float32)
    b1 = np.asarray(b1, dtype=np.float32)
    W2 = np.asarray(W2, dtype=np.float32)
    b2 = np.asarray(b2, dtype=np.float32)
    qt = np.asarray(question_types)
    N = x.shape[0]

    idx = [np.nonzero(qt == e)[0] for e in range(E)]
    counts = [len(i) for i in idx]
    S = max(int(np.ceil(max(counts) / 16) * 16), 4 * CHUNK)

    nc = _program_cache.get(S)
    if nc is None:
        nc = _build_program(S)
        _program_cache[S] = nc

    x16 = x.astype(BF16)
    spans = _x_spans(S)
    in_maps = []
    for e in range(E):
        cnt = counts[e]
        xt_f = np.zeros((P, DBLK, S), BF16)
        xt_f[:, :, :cnt] = x16[idx[e]].reshape(cnt, DBLK, P).transpose(2, 1, 0)
        # span-contiguous repack: [P, sum_over_spans(DBLK*n)]
        xt = np.concatenate(
            [xt_f[:, :, off : off + n].reshape(P, DBLK * n) for off, n in spans],
            axis=1,
        )
        # hb-major layout so the first 768 cols (= hb0's six d-blocks) can
        # be DMA'd first and gate only the first chunk's matmuls
        w1t = (
            W1[e]
            .reshape(DBLK, P, HBLK, P)
            .transpose(1, 2, 0, 3)
            .reshape(P, DBLK * H)
        )
        w2t = W2[e].reshape(HBLK, P, C).transpose(1, 0, 2).reshape(P, HBLK * C)
        wt = np.concatenate([w1t, w2t], axis=1).astype(BF16)
        bt = b1[e].reshape(HBLK, P).T.copy()
        in_maps.append({"xt": xt, "wt": wt, "bt": bt})

    r = run_bass_kernel_spmd(nc, in_maps, list(range(NCORES)))
    last_results = r

    # the final chunk's columns hold the already-folded result in rows 0-7
    last_soff, last_sn = spans[-1]
    fin = last_soff + ((last_sn - 1) // CHUNK) * CHUNK
    out = np.zeros((N, C), np.float32)
    for e in range(E):
        p = r.results[e]["yt"]
        y = p[0:C] + p[32 : 32 + C] + p[64 : 64 + C]
        y[:, fin:] = p[0:C, fin:]
        y += b2[e][:, None]
        out[idx[e]] = y[:, : counts[e]].T
    return out
